# revision 17
# baseline (speedup 1.0000x reference)
"""Causal self-attention (B=4, T=2048, C=768, 12 heads) on 8 Trainium2 NeuronCores.

Sharding: core c -> batch b = c//2, head group hg = c%2 (6 heads each).
Flipped-AV flash attention: scores^T = K^T-chunk @ Q (two heads row-packed in
the PE), exp on ACT (bias -2, fp8e4 out), AV as out^T[d(+ones),q] =
v65.T @ ex with fp8 DoubleRow pairing adjacent k-blocks, row-sums ride the
ones column, softmax normalize via PE rc-broadcast + DVE multiply writing
y^T directly in proj-lhsT layout (no transposes), then output projection.
Host sums the two half-head partial y's per batch and adds b_proj.
"""

import numpy as np

import concourse.bacc as bacc
import concourse.bass as bass
import concourse.mybir as mybir
import concourse.tile as tile

B, T, C = 4, 2048, 768
NH, HD = 12, 64
NHL = 6            # heads per core
N_CORES = 8
TQ = 512           # q supertile width
NTB = T // 128     # 16 token blocks

F32 = mybir.dt.float32
FP8 = mybir.dt.float8e4
DRMODE = mybir.MatmulPerfMode.DoubleRow

AV_FP8 = True      # fp8e4 + DoubleRow for the AV matmuls (else bf16)
VW = 80            # padded per-head v width (fp8 DR needs 16B-aligned kb stride)


def _build_program(cdt=mybir.dt.bfloat16, n_iters=1, with_bqk=False, with_bv=False):
    """Build the SPMD single-core program. Returns nc."""
    nc = bacc.Bacc("TRN2", target_bir_lowering=False, debug=False,
                   num_devices=N_CORES)
    exdt = FP8 if AV_FP8 else cdt
    vdt = FP8 if AV_FP8 else cdt

    xT_d = nc.dram_tensor("xT", [C, T], cdt, kind="ExternalInput")
    wqk_d = nc.dram_tensor("wqk", [C, 768], cdt, kind="ExternalInput")
    wv_d = nc.dram_tensor("wv", [C, 384], cdt, kind="ExternalInput")
    wp_d = nc.dram_tensor("wp", [384, C], cdt, kind="ExternalInput")
    mb_d = nc.dram_tensor("maskbias", [128, 2, 128], F32, kind="ExternalInput")
    if with_bqk:
        bqk_d = nc.dram_tensor("bqk", [C, 1], F32, kind="ExternalInput")
    if with_bv:
        bv_d = nc.dram_tensor("bv", [128, 384], cdt, kind="ExternalInput")
    y_d = nc.dram_tensor("y", [T, C], F32, kind="ExternalOutput")

    with tile.TileContext(nc) as tc:
        with (
            tc.tile_pool(name="consts", bufs=1) as consts,
            tc.tile_pool(name="big", bufs=1) as big,
            tc.tile_pool(name="work", bufs=3) as work,
            tc.tile_pool(name="expool", bufs=8) as expool,
            tc.tile_pool(name="small", bufs=8) as small,
            tc.tile_pool(name="psA", bufs=2, space="PSUM") as psA,
            tc.tile_pool(name="psB", bufs=3, space="PSUM") as psB,
            tc.tile_pool(name="psP", bufs=1, space="PSUM") as psP,
        ):
            xT = consts.tile([128, 6, T], cdt)
            wqk = consts.tile([128, 6, 768], cdt)
            wv = consts.tile([128, 6, 384], cdt)
            wp = consts.tile([128, 3, 768], cdt)
            maskb = consts.tile([128, 2, 128], F32)
            bias2 = consts.tile([128, 1], F32)
            if with_bqk:
                bqk = consts.tile([128, 6, 1], F32)
            if with_bv:
                bv = consts.tile([128, 384], cdt)

            qt_sb = big.tile([128, 3, T], cdt)
            kt_sb = big.tile([128, 3, T], cdt)
            v_sb = big.tile([128, NTB, NHL, 2, VW], vdt)
            attT_sb = big.tile([128, 3, T], cdt)

            def body():
                xT_r = xT_d.rearrange("(n p) t -> p n t", p=128)
                wqk_r = wqk_d.rearrange("(n p) m -> p n m", p=128)
                wv_r = wv_d.rearrange("(n p) m -> p n m", p=128)
                for kc in range(6):
                    nc.sync.dma_start(xT[:, kc, :], xT_r[:, kc, :])
                    nc.sync.dma_start(wqk[:, kc, :], wqk_r[:, kc, :])
                    nc.sync.dma_start(wv[:, kc, :], wv_r[:, kc, :])
                nc.sync.dma_start(wp[:], wp_d.rearrange("(n p) m -> p n m", p=128))
                nc.sync.dma_start(maskb[:], mb_d[:])
                nc.gpsimd.memset(bias2[:], -2.0)
                if with_bqk:
                    nc.sync.dma_start(bqk[:], bqk_d.rearrange("(n p) o -> p n o", p=128))
                if with_bv:
                    nc.sync.dma_start(bv[:], bv_d[:])

                nc.gpsimd.memset(v_sb[:, :, :, 0, HD:HD + 1], 1.0)
                if AV_FP8:
                    nc.gpsimd.memset(v_sb[:, :, :, 1, HD:HD + 1], 0.0)

                def qk_group(nt, j):
                    """One q/k 128-col supertile group of QKV."""
                    nw = slice(nt * 512, (nt + 1) * 512)
                    dst = qt_sb if j < 3 else kt_sb
                    jj = j % 3
                    ps = psA.tile([128, 2, 512], F32, tag="A")
                    for kc in range(6):
                        nc.tensor.matmul(
                            ps[:, 0, :],
                            lhsT=wqk[:, kc, j * 128:(j + 1) * 128],
                            rhs=xT[:, kc, nw],
                            start=(kc == 0), stop=(kc == 5),
                        )
                    if with_bqk:
                        nc.vector.tensor_scalar_add(
                            dst[:, jj, nw], ps[:, 0, :],
                            bqk[:, jj if j < 3 else jj + 3])
                    else:
                        nc.vector.tensor_copy(dst[:, jj, nw], ps[:, 0, :])

                def v_group(tb):
                    """One v token-block of QKV."""
                    ps = psA.tile([128, 2, 512], F32, tag="A", name="psv")
                    for kc in range(6):
                        nc.tensor.matmul(
                            ps[:, 0, 0:384],
                            lhsT=xT[:, kc, tb * 128:(tb + 1) * 128],
                            rhs=wv[:, kc, :],
                            start=(kc == 0), stop=(kc == 5),
                        )
                    psr = ps[:, 0, 0:384].rearrange("p (h d) -> p h d", h=NHL)
                    if with_bv:
                        vtmp = work.tile([128, 6, 64], cdt, tag="vtmp")
                        nc.vector.tensor_add(
                            vtmp[:], psr,
                            bv.rearrange("p (h d) -> p h d", h=NHL))
                        nc.vector.tensor_copy(v_sb[:, tb, :, 0, 0:HD], vtmp[:])
                    else:
                        nc.vector.tensor_copy(v_sb[:, tb, :, 0, 0:HD], psr)
                    if AV_FP8:  # residual: v_lo = v - fp8(v), quantized again
                        nc.vector.scalar_tensor_tensor(
                            v_sb[:, tb, :, 1, 0:HD], psr, 1.0,
                            v_sb[:, tb, :, 0, 0:HD],
                            mybir.AluOpType.bypass,
                            mybir.AluOpType.subtract)

                def scores_pair(qt, hp, p, ex):
                    """Scores + mask + exp for k-block pair p of (qt, hp)."""
                    qw = slice(qt * TQ, (qt + 1) * TQ)
                    for kk in range(2):
                        kb = 2 * p + kk
                        j = kb - 4 * qt  # diag block index if >= 0
                        sc = psA.tile([128, 2, 512], F32, tag="A", name="sc")
                        kw = slice(kb * 128, (kb + 1) * 128)
                        for h01 in range(2):
                            pb = h01 * 64
                            nc.tensor.matmul(
                                sc[:, h01, :],
                                lhsT=kt_sb[pb:pb + 64, hp, kw],
                                rhs=qt_sb[pb:pb + 64, hp, qw],
                                start=True, stop=True,
                            )
                        if j >= 0:  # diagonal: additive mask pre-exp
                            nc.vector.scalar_tensor_tensor(
                                sc[:, :, j * 128:(j + 1) * 128],
                                sc[:, :, j * 128:(j + 1) * 128],
                                1.0, maskb[:],
                                mybir.AluOpType.bypass,
                                mybir.AluOpType.add)
                            lo = j * 128
                            if lo > 0:  # zero above-diag ex region
                                nc.gpsimd.memset(ex[:, :, kk, 0:lo], 0.0)
                        else:
                            lo = 0
                        nc.scalar.activation(
                            out=ex[:, :, kk, lo:512],
                            in_=sc[:, :, lo:512],
                            func=mybir.ActivationFunctionType.Exp,
                            bias=bias2[:], scale=1.0)

                def av_pair(qt, hp, p, ex, tX):
                    npair = 2 * qt + 2
                    for h01 in range(2):
                        h = 2 * hp + h01
                        if AV_FP8:
                            for pl in range(2):  # hi then lo residual plane
                                nc.tensor.matmul(
                                    tX[h01][0:65, :],
                                    lhsT=v_sb[:, 2 * p:2 * p + 2, h, pl, 0:65],
                                    rhs=ex[:, h01, :, :],
                                    start=(p == 0 and pl == 0),
                                    stop=(p == npair - 1 and pl == 1),
                                    perf_mode=DRMODE,
                                )
                        else:
                            for kk in range(2):
                                nc.tensor.matmul(
                                    tX[h01][0:65, :],
                                    lhsT=v_sb[:, 2 * p + kk, h, 0:65],
                                    rhs=ex[:, h01, kk, :],
                                    start=(p == 0 and kk == 0),
                                    stop=(p == npair - 1 and kk == 1),
                                )

                def make_norm(qt, hp, tX):
                    """Softmax normalize -> attT in proj-lhsT layout."""
                    def norm():
                        qw = slice(qt * TQ, (qt + 1) * TQ)
                        rcrow = small.tile([1, 2, 512], F32, tag="rc")
                        for h01 in range(2):
                            nc.vector.reciprocal(
                                rcrow[:, h01, :], tX[h01][64:65, :])
                        for h01 in range(2):
                            rcb = small.tile([64, 512], F32, tag=f"rcb{h01}",
                                             name="rcb")
                            nc.gpsimd.partition_broadcast(
                                rcb[:], rcrow[:, h01, :], channels=64)
                            nc.vector.scalar_tensor_tensor(
                                attT_sb[64 * h01:64 * h01 + 64, hp, qw],
                                tX[h01][0:64, :],
                                1.0, rcb[:],
                                mybir.AluOpType.bypass,
                                mybir.AluOpType.mult)
                    return norm

                def proj_tb(tb, last):
                    tw = slice(tb * 128, (tb + 1) * 128)
                    if last:  # sc pool is free: pipelined 2-bank tiles
                        pa = psA.tile([128, 2, 512], F32, tag="A", name="pa")
                        g1, g2 = pa[:, 0, :], pa[:, 1, 0:256]
                    else:     # 1-bank tile; trails behind next-qt scores
                        pa = psP.tile([128, 512], F32, tag="P", name="pp")
                        g1, g2 = pa[:, :], pa[:, 0:256]
                    ysb = work.tile([128, 768], F32, tag="ysb")
                    for hp in range(3):
                        nc.tensor.matmul(
                            g1, lhsT=attT_sb[:, hp, tw],
                            rhs=wp[:, hp, 0:512],
                            start=(hp == 0), stop=(hp == 2))
                    nc.vector.tensor_copy(ysb[:, 0:512], g1)
                    for hp in range(3):
                        nc.tensor.matmul(
                            g2, lhsT=attT_sb[:, hp, tw],
                            rhs=wp[:, hp, 512:768],
                            start=(hp == 0), stop=(hp == 2))
                    nc.vector.tensor_copy(ysb[:, 512:768], g2)
                    nc.sync.dma_start(y_d[tw, :], ysb[:])

                # ---- software-pipelined attention + interleaved fillers ----
                # PE order per pair-slot: scores(p) ... AV(p-1); one pending
                # normalize flushes after the first scores of the next hp;
                # proj(qt-1) and QKV chunk(qt+1) pieces fill spare slots.
                for j in (3, 4, 5, 0, 1, 2):
                    qk_group(0, j)
                for tb in range(4):
                    v_group(tb)

                NQT = T // TQ
                pend_norm = None
                for qt in range(NQT):
                    # filler units for this qt: proj of qt-1, then chunk qt+1
                    fillers = []
                    if qt > 0:
                        fillers += [lambda tb=tb: proj_tb(tb, False)
                                    for tb in range((qt - 1) * 4, qt * 4)]
                    if qt + 1 < NQT:
                        fillers += [lambda j=j: qk_group(qt + 1, j)
                                    for j in (3, 4, 5, 0, 1, 2)]
                        fillers += [lambda tb=tb: v_group(tb)
                                    for tb in range(4 * qt + 4, 4 * qt + 8)]
                    nslots = 3 * (2 * qt + 2)
                    per_slot = [len(fillers) * (s + 1) // nslots -
                                len(fillers) * s // nslots
                                for s in range(nslots)]
                    slot = 0
                    for hp in range(3):
                        npair = 2 * qt + 2
                        tX = [psB.tile([128, 512], F32, tag="B", name=f"t{h01}")
                              for h01 in range(2)]
                        exs = {}
                        for p in range(npair):
                            exs[p] = expool.tile([128, 2, 2, 512], exdt,
                                                 tag="ex", name="ex")
                            scores_pair(qt, hp, p, exs[p])
                            if p == 0 and pend_norm is not None:
                                pend_norm()
                                pend_norm = None
                            if p >= 1:
                                av_pair(qt, hp, p - 1, exs[p - 1], tX)
                                del exs[p - 1]
                            for _ in range(per_slot[slot]):
                                fillers.pop(0)()
                            slot += 1
                        av_pair(qt, hp, npair - 1, exs[npair - 1], tX)
                        exs.clear()
                        pend_norm = make_norm(qt, hp, tX)

                # tail: last normalize + projection of the last supertile
                pend_norm()
                for tb in range((NQT - 1) * 4, NQT * 4):
                    proj_tb(tb, True)

            if n_iters == 1:
                body()
            else:
                with tc.For_i(0, n_iters, 1,
                              hint_engines=(mybir.EngineType.PE,
                                            mybir.EngineType.DVE,
                                            mybir.EngineType.Activation)):
                    body()

    nc.compile()
    return nc


def _host_prep(inputs, cdt_np):
    """Per-core input maps from full inputs."""
    x = np.asarray(inputs["x"], np.float32)
    w_attn = np.asarray(inputs["w_attn"], np.float32)
    b_attn = np.asarray(inputs["b_attn"], np.float32)
    w_proj = np.asarray(inputs["w_proj"], np.float32)

    kk = np.arange(128)
    maskbias = np.where(kk[:, None] <= kk[None, :], 0.0, -30000.0)
    maskbias = np.broadcast_to(maskbias[:, None, :], (128, 2, 128))
    maskbias = np.ascontiguousarray(maskbias, dtype=np.float32)
    with_bqk = bool(np.any(b_attn[0:1536] != 0))
    with_bv = bool(np.any(b_attn[1536:2304] != 0))

    in_maps = []
    for c in range(N_CORES):
        b, hg = c // 2, c % 2
        cols = slice(hg * 384, hg * 384 + 384)
        wq = w_attn[:, 0:768][:, cols] * 0.125
        wk = w_attn[:, 768:1536][:, cols]
        m = {
            "xT": np.ascontiguousarray(x[b].T).astype(cdt_np),
            "wqk": np.concatenate([wq, wk], axis=1).astype(cdt_np),
            "wv": np.ascontiguousarray(w_attn[:, 1536:2304][:, cols]).astype(cdt_np),
            "wp": np.ascontiguousarray(w_proj[cols, :]).astype(cdt_np),
            "maskbias": maskbias,
        }
        if with_bqk:
            bq = b_attn[0:768][cols] * 0.125
            bk = b_attn[768:1536][cols]
            m["bqk"] = np.concatenate([bq, bk]).astype(np.float32).reshape(C, 1)
        if with_bv:
            bvv = b_attn[1536:2304][cols].astype(cdt_np)
            m["bv"] = np.broadcast_to(bvv, (128, 384)).copy()
        in_maps.append(m)
    return in_maps, with_bqk, with_bv


_CACHE = {}


def _get_runner(cdt, n_iters, with_bqk, with_bv, donate=True):
    """Build program + persistent jitted PJRT callable (cached)."""
    key = (str(cdt), n_iters, with_bqk, with_bv, donate)
    if key in _CACHE:
        return _CACHE[key]

    import jax
    from jax.sharding import Mesh, PartitionSpec
    from jax.experimental.shard_map import shard_map
    from concourse.bass2jax import (_bass_exec_p, install_neuronx_cc_hook,
                                    partition_id_tensor)

    nc = _build_program(cdt=cdt, n_iters=n_iters,
                        with_bqk=with_bqk, with_bv=with_bv)
    install_neuronx_cc_hook()

    partition_name = nc.partition_id_tensor.name if nc.partition_id_tensor else None
    in_names, out_names, out_avals = [], [], []
    for alloc in nc.m.functions[0].allocations:
        if not isinstance(alloc, mybir.MemoryLocationSet):
            continue
        name = alloc.memorylocations[0].name
        if alloc.kind == "ExternalInput":
            if name != partition_name:
                in_names.append(name)
        elif alloc.kind == "ExternalOutput":
            out_names.append(name)
            out_avals.append(jax.core.ShapedArray(
                tuple(alloc.tensor_shape), mybir.dt.np(alloc.dtype)))
    n_params = len(in_names)
    n_outs = len(out_avals)
    all_names = list(in_names) + list(out_names)
    if partition_name is not None:
        all_names.append(partition_name)
    donate_ = tuple(range(n_params, n_params + n_outs))

    def _bodyfn(*args):
        operands = list(args)
        if partition_name is not None:
            operands.append(partition_id_tensor())
        outs = _bass_exec_p.bind(
            *operands,
            out_avals=tuple(out_avals),
            in_names=tuple(all_names),
            out_names=tuple(out_names),
            lowering_input_output_aliases=(),
            sim_require_finite=True,
            sim_require_nnan=True,
            nc=nc,
        )
        return tuple(outs)

    devices = jax.devices()[:N_CORES]
    mesh = Mesh(np.asarray(devices), ("core",))
    in_specs = (PartitionSpec("core"),) * (n_params + n_outs)
    out_specs = (PartitionSpec("core"),) * n_outs
    fn = jax.jit(
        shard_map(_bodyfn, mesh=mesh, in_specs=in_specs, out_specs=out_specs,
                  check_rep=False),
        donate_argnums=donate_ if donate else (), keep_unused=True)

    runner = (fn, in_names, out_names, out_avals)
    _CACHE[key] = runner
    return runner


def _run(in_maps, cdt, n_iters, with_bqk, with_bv):
    import jax
    fn, in_names, out_names, out_avals = _get_runner(cdt, n_iters, with_bqk, with_bv)
    concat_in = [np.concatenate([m[nm] for m in in_maps], axis=0)
                 for nm in in_names]
    zeros = [np.zeros((N_CORES * av.shape[0], *av.shape[1:]), av.dtype)
             for av in out_avals]
    outs = fn(*concat_in, *zeros)
    jax.block_until_ready(outs)
    y = np.asarray(outs[out_names.index("y")]).reshape(N_CORES, T, C)
    return y


def kernel(**inputs) -> np.ndarray:
    import ml_dtypes
    cdt, cdt_np = mybir.dt.bfloat16, ml_dtypes.bfloat16
    in_maps, with_bqk, with_bv = _host_prep(inputs, cdt_np)
    y_parts = _run(in_maps, cdt, 1, with_bqk, with_bv)

    b_proj = np.asarray(inputs["b_proj"], np.float32)
    out = np.empty((B, T, C), np.float32)
    for b in range(B):
        out[b] = y_parts[2 * b] + y_parts[2 * b + 1] + b_proj
    return out
